# revision 20
# baseline (speedup 1.0000x reference)
"""Trainium2 Bass kernel for ragged bag-attention (nn_Attention).

Algorithm (per sentence i, bag b): logit_i = <x_i, att[q_i]*rel[q_i]>;
w = softmax(logit) within bag; bag_repr_b = sum w_i x_i; out = bag_repr @ rel.T + bias.

Device strategy (8 cores, sentence-sharded):
  - Sentences packed into 128-row chunks with <=16 bag-fragments per chunk
    (bags may split across chunks/cores; fragment partial sums are combined
    on host, exploiting exp(logit) being max-free safe: |logit| < ~0.5).
  - Per chunk: G = onehotT(q).T @ (att*rel)  (PE, fp32r)
               logit = rowsum(x * G)         (DVE tensor_tensor_reduce)
               e = exp(logit + pad_bias)     (ACT)
               E^T[i,j] = (j==relseg_i)*e_i  (DVE tensor_scalar)
               frag_sums = E^T.T @ [x|1]     (PE, fp32r -> PSUM)
  - Every 8 chunks the PSUM fragment table [128, 691] is copied to SBUF (DVE)
    and DMA'd out. Host: U = table @ rel.T, bin by bag, divide by denom, +bias.
"""
import sys
sys.path.insert(0, '/opt/trn_rl_repo')
import numpy as np

NCORES = 8
DIM = 690
NCLS = 53
CHUNK = 128
BSLOT = 16
GROUP = 4           # chunks per PSUM flush group

_cache = {}         # nchunk -> compiled Bass module


def _pack_core(scope, seg, lo, hi):
    """Pack sentences [lo,hi) into chunks of <=CHUNK sentences and <=BSLOT
    bag-fragments. Returns list of chunks, each a list of (bag, start, take)."""
    b0, b1 = int(seg[lo]), int(seg[hi - 1])
    chunks, cur, fill = [], [], 0
    for b in range(b0, b1 + 1):
        s = max(int(scope[b]), lo)
        e = min(int(scope[b + 1]), hi)
        m = e - s
        while m > 0:
            if fill == CHUNK or len(cur) == BSLOT:
                chunks.append(cur)
                cur, fill = [], 0
            take = min(m, CHUNK - fill)
            cur.append((b, s, take))
            fill += take
            s += take
            m -= take
    if cur:
        chunks.append(cur)
    return chunks


def _build_module(nchunk):
    from concourse import bacc, mybir
    from concourse.tile import TileContext

    f32 = mybir.dt.float32
    f32r = mybir.dt.float32r
    S = nchunk * CHUNK
    W = DIM + 2          # 692 padded row width
    assert nchunk % 8 == 0
    groups = nchunk // GROUP

    nc = bacc.Bacc()
    bf16 = mybir.dt.bfloat16
    # xp is host-preblocked: row (tb*128+p) holds the 4 chunk-rows
    # {512tb+128u+p : u<4} side by side -> one 11KB descriptor per partition.
    xp_d = nc.declare_dram_parameter("xp", [(nchunk // 4) * CHUNK, 4 * W], f32r,
                                     isOutput=False)
    oh_d = nc.declare_dram_parameter("oh", [NCLS, S], bf16, isOutput=False)
    cw_d = nc.declare_dram_parameter("cw", [NCLS, DIM], bf16, isOutput=False)
    rs_d = nc.declare_dram_parameter("rs", [CHUNK, nchunk], f32, isOutput=False)
    io_d = nc.declare_dram_parameter("io32", [CHUNK, 2 * BSLOT], f32, isOutput=False)
    tab_d = nc.declare_dram_parameter("tab", [nchunk * BSLOT, W], f32,
                                      isOutput=True)

    with TileContext(nc) as tc:
        with (
            tc.tile_pool(name="consts", bufs=1) as cpool,
            tc.tile_pool(name="xb", bufs=4) as xpool,
            tc.tile_pool(name="prod", bufs=2) as ppool,
            tc.tile_pool(name="small", bufs=4) as spool,
            tc.tile_pool(name="flush", bufs=2) as fpool,
            tc.tile_pool(name="gps", bufs=2, space="PSUM") as gpool,
            tc.tile_pool(name="bags", bufs=2, space="PSUM") as bpool,
        ):
            oh_sb = cpool.tile([NCLS, S], bf16)
            for q in range(4):
                nc.scalar.dma_start(out=oh_sb[:, q * S // 4:(q + 1) * S // 4],
                                    in_=oh_d[:, q * S // 4:(q + 1) * S // 4])
            cw_sb = cpool.tile([NCLS, DIM], bf16)
            nc.scalar.dma_start(out=cw_sb[:, :], in_=cw_d[:, :])
            rs_sb = cpool.tile([CHUNK, nchunk], f32)
            nc.scalar.dma_start(out=rs_sb[:, :], in_=rs_d[:, :])
            io_sb = cpool.tile([CHUNK, 2 * BSLOT], f32)
            nc.scalar.dma_start(out=io_sb[:, :], in_=io_d[:, :])

            fl = None
            for tb in range(nchunk // 4):
                # one DMA loads 4 chunks: DRAM rows (u p) -> SBUF [p, u*W:(u+1)*W]
                xb = xpool.tile([CHUNK, 4 * W], f32r)
                nc.sync.dma_start(
                    out=xb[:, :],
                    in_=xp_d[tb * CHUNK:(tb + 1) * CHUNK, :])
                for u4 in range(4):
                    t = tb * 4 + u4
                    xe = xb[:, u4 * W:(u4 + 1) * W]
                    if t % 2 == 0:
                        bag = bpool.tile([32, 1024], f32)  # [0:346],[512:858]

                    G = gpool.tile([CHUNK, 1024], f32)    # [0:346],[512:856]
                    ohT = oh_sb[:, t * CHUNK:(t + 1) * CHUNK]
                    nc.tensor.matmul(G[:, 0:346], ohT, cw_sb[:, 0:346],
                                     start=True, stop=True)
                    nc.tensor.matmul(G[:, 512:856], ohT, cw_sb[:, 346:DIM],
                                     start=True, stop=True)

                    prod = ppool.tile([CHUNK, DIM], f32)
                    la = spool.tile([CHUNK, 1], f32)
                    lb2 = spool.tile([CHUNK, 1], f32)
                    xv = xe.bitcast(f32)
                    nc.vector.affine_mul_reduce(
                        out=prod[:, 0:346], accum_out=la[:, 0:1],
                        in0=xv[:, 0:346], in1=G[:, 0:346], scale=1.0, bias=0.0)
                    nc.vector.affine_mul_reduce(
                        out=prod[:, 346:DIM], accum_out=lb2[:, 0:1],
                        in0=xv[:, 346:DIM], in1=G[:, 512:856], scale=1.0, bias=0.0)

                    # e = exp(la + lb2); pad rows are all-zero in xe (incl the
                    # ones column) so their e value is irrelevant.
                    e = spool.tile([CHUNK, 1], f32)
                    nc.scalar.activation(e[:, 0:1], la[:, 0:1],
                                         mybir.ActivationFunctionType.Exp,
                                         bias=lb2[:, 0:1], scale=1.0)

                    # two consecutive chunks share one 32-row PSUM block:
                    # even chunk slots 0:16, odd chunk slots 16:32 (host adds
                    # 16 to relseg of odd chunks), accumulated via start/stop.
                    ET = spool.tile([CHUNK, 2 * BSLOT], f32r)
                    nc.vector.tensor_scalar(
                        out=ET[:, :], in0=io_sb[:, :], scalar1=rs_sb[:, t:t + 1],
                        scalar2=e[:, 0:1], op0=mybir.AluOpType.is_equal,
                        op1=mybir.AluOpType.mult)

                    first = (t % 2 == 0)
                    nc.tensor.matmul(bag[0:32, 0:346], ET[:, :], xe[:, 0:346],
                                     start=first, stop=not first)
                    nc.tensor.matmul(bag[0:32, 512:858], ET[:, :],
                                     xe[:, 346:W], start=first, stop=not first)

                    if t % 2 == 1:
                        p = t // 2
                        if p % 4 == 0:
                            fl = fpool.tile([32, 4 * W], f32)
                        # one copy per pair: both PSUM banks via 3D AP
                        nc.scalar.copy(
                            out=fl[:, (p % 4) * W:(p % 4) * W + 692]
                                .rearrange("q (a b) -> q a b", a=2, b=346),
                            in_=bag[0:32, 0:1024]
                                .rearrange("q (a b) -> q a b", a=2, b=512)
                                [:, :, 0:346])
                        if p % 4 == 3:
                            q4 = p // 4
                            dst = tab_d[q4 * 4 * 32:(q4 + 1) * 4 * 32, :]
                            nc.scalar.dma_start(
                                out=dst.rearrange("(u q) d -> q u d", u=4),
                                in_=fl[:, :].rearrange("q (u d) -> q u d", u=4))

    nc.compile()
    return nc


def _prepare(x, rel_weight, att_weight, bias, attention_query, scope):
    x = np.asarray(x, dtype=np.float32)
    rel_weight = np.asarray(rel_weight, dtype=np.float32)
    att_weight = np.asarray(att_weight, dtype=np.float32)
    bias = np.asarray(bias, dtype=np.float32)
    q = np.asarray(attention_query).astype(np.int64)
    scope = np.asarray(scope).astype(np.int64)

    nsent = x.shape[0]
    nbags = len(scope) - 1
    score = nsent // NCORES
    seg = (np.searchsorted(scope, np.arange(nsent), side='right') - 1)
    import ml_dtypes
    cw = (att_weight * rel_weight).astype(ml_dtypes.bfloat16)

    all_chunks = [_pack_core(scope, seg, c * score, (c + 1) * score)
                  for c in range(NCORES)]
    nchunk = max(len(ch) for ch in all_chunks)
    nchunk = (nchunk + 7) // 8 * 8      # device loop needs a multiple of 8
    S = nchunk * CHUNK

    import ml_dtypes
    iota32 = np.ascontiguousarray(
        np.broadcast_to(np.arange(2 * BSLOT, dtype=np.float32), (CHUNK, 2 * BSLOT)))
    in_maps = []
    frag2bag = []
    for c in range(NCORES):
        idx = np.full(S, -1, np.int64)
        relseg = np.zeros(S, np.float32)
        f2b = np.full((nchunk, BSLOT), -1, np.int64)
        for k, ch in enumerate(all_chunks[c]):
            p = k * CHUNK
            for j, (b, s, take) in enumerate(ch):
                idx[p:p + take] = np.arange(s, s + take)
                relseg[p:p + take] = j + BSLOT * (k % 2)
                f2b[k, j] = b
                p += take
        valid = idx >= 0
        xp = np.zeros((S, DIM + 2), np.float32)
        xp[valid, DIM] = 1.0
        xp[valid, :DIM] = x[idx[valid]]
        # pre-block: [nblocks, 4, 128, W] -> [nblocks, 128, 4, W] flat
        xp = np.ascontiguousarray(
            xp.reshape(nchunk // 4, 4, CHUNK, DIM + 2).transpose(0, 2, 1, 3)
        ).reshape((nchunk // 4) * CHUNK, 4 * (DIM + 2))
        qp = np.zeros(S, np.int64)
        qp[valid] = q[idx[valid]]
        oh = (qp[None, :] == np.arange(NCLS)[:, None]).astype(ml_dtypes.bfloat16)
        in_maps.append({
            "xp": xp,
            "oh": np.ascontiguousarray(oh),
            "cw": cw,
            "rs": np.ascontiguousarray(relseg.reshape(nchunk, CHUNK).T),
            "io32": iota32,
        })
        frag2bag.append(f2b)
    return in_maps, frag2bag, nchunk, nbags, rel_weight, bias


def _assemble(tables, frag2bag, nchunk, nbags, rel_weight, bias):
    num = np.zeros((nbags, NCLS))
    den = np.zeros(nbags)
    for c in range(NCORES):
        table = tables[c].reshape(nchunk * BSLOT, DIM + 2)
        U = table[:, :DIM] @ rel_weight.T
        d = table[:, DIM]
        fb = frag2bag[c].ravel()
        v = fb >= 0
        for k in range(NCLS):
            num[:, k] += np.bincount(fb[v], U[v, k], minlength=nbags)
        den += np.bincount(fb[v], d[v], minlength=nbags)
    return (num / den[:, None] + bias[None, :]).astype(np.float32)


def kernel(x, rel_weight, att_weight, bias, attention_query, scope):
    from concourse.bass_utils import run_bass_kernel_spmd

    in_maps, frag2bag, nchunk, nbags, rel, b = _prepare(
        x, rel_weight, att_weight, bias, attention_query, scope)
    if nchunk not in _cache:
        _cache[nchunk] = _build_module(nchunk)
    nc = _cache[nchunk]
    res = run_bass_kernel_spmd(nc, in_maps, list(range(NCORES)))
    tables = [res.results[c]["tab"] for c in range(NCORES)]
    return _assemble(tables, frag2bag, nchunk, nbags, rel, b)


# revision 21
# speedup vs baseline: 1.0092x; 1.0092x over previous
"""Trainium2 Bass kernel for ragged bag-attention (nn_Attention).

Algorithm (per sentence i, bag b): logit_i = <x_i, att[q_i]*rel[q_i]>;
w = softmax(logit) within bag; bag_repr_b = sum w_i x_i; out = bag_repr @ rel.T + bias.

Device strategy (8 cores, sentence-sharded):
  - Sentences packed into 128-row chunks with <=16 bag-fragments per chunk
    (bags may split across chunks/cores; fragment partial sums are combined
    on host, exploiting exp(logit) being max-free safe: |logit| < ~0.5).
  - Per chunk: G = onehotT(q).T @ (att*rel)  (PE, fp32r)
               logit = rowsum(x * G)         (DVE tensor_tensor_reduce)
               e = exp(logit + pad_bias)     (ACT)
               E^T[i,j] = (j==relseg_i)*e_i  (DVE tensor_scalar)
               frag_sums = E^T.T @ [x|1]     (PE, fp32r -> PSUM)
  - Every 8 chunks the PSUM fragment table [128, 691] is copied to SBUF (DVE)
    and DMA'd out. Host: U = table @ rel.T, bin by bag, divide by denom, +bias.
"""
import sys
sys.path.insert(0, '/opt/trn_rl_repo')
import numpy as np

NCORES = 8
DIM = 690
NCLS = 53
CHUNK = 128
BSLOT = 16
GROUP = 4           # chunks per PSUM flush group

_cache = {}         # nchunk -> compiled Bass module


def _pack_core(scope, seg, lo, hi):
    """Pack sentences [lo,hi) into chunks of <=CHUNK sentences and <=BSLOT
    bag-fragments. Returns list of chunks, each a list of (bag, start, take)."""
    b0, b1 = int(seg[lo]), int(seg[hi - 1])
    chunks, cur, fill = [], [], 0
    for b in range(b0, b1 + 1):
        s = max(int(scope[b]), lo)
        e = min(int(scope[b + 1]), hi)
        m = e - s
        while m > 0:
            if fill == CHUNK or len(cur) == BSLOT:
                chunks.append(cur)
                cur, fill = [], 0
            take = min(m, CHUNK - fill)
            cur.append((b, s, take))
            fill += take
            s += take
            m -= take
    if cur:
        chunks.append(cur)
    return chunks


def _build_module(nchunk):
    from concourse import bacc, mybir
    from concourse.tile import TileContext

    f32 = mybir.dt.float32
    f32r = mybir.dt.float32r
    S = nchunk * CHUNK
    W = DIM + 2          # 692 padded row width
    assert nchunk % 8 == 0
    groups = nchunk // GROUP

    nc = bacc.Bacc()
    bf16 = mybir.dt.bfloat16
    # xp is host-preblocked: row (tb*128+p) holds the 4 chunk-rows
    # {512tb+128u+p : u<4} side by side -> one 11KB descriptor per partition.
    xp_d = nc.declare_dram_parameter("xp", [(nchunk // 4) * CHUNK, 4 * W], f32r,
                                     isOutput=False)
    oh_d = nc.declare_dram_parameter("oh", [NCLS, S], bf16, isOutput=False)
    cw_d = nc.declare_dram_parameter("cw", [NCLS, DIM], bf16, isOutput=False)
    rs_d = nc.declare_dram_parameter("rs", [CHUNK, nchunk], f32, isOutput=False)
    io_d = nc.declare_dram_parameter("io32", [CHUNK, 2 * BSLOT], f32, isOutput=False)
    tab_d = nc.declare_dram_parameter("tab", [nchunk * BSLOT, W], f32,
                                      isOutput=True)

    with TileContext(nc) as tc:
        with (
            tc.tile_pool(name="consts", bufs=1) as cpool,
            tc.tile_pool(name="xb", bufs=4) as xpool,
            tc.tile_pool(name="prod", bufs=2) as ppool,
            tc.tile_pool(name="small", bufs=4) as spool,
            tc.tile_pool(name="flush", bufs=2) as fpool,
            tc.tile_pool(name="gps", bufs=2, space="PSUM") as gpool,
            tc.tile_pool(name="bags", bufs=2, space="PSUM") as bpool,
        ):
            oh_sb = cpool.tile([NCLS, S], bf16)
            nc.scalar.dma_start(out=oh_sb[:, :], in_=oh_d[:, :])
            cw_sb = cpool.tile([NCLS, DIM], bf16)
            nc.scalar.dma_start(out=cw_sb[:, :], in_=cw_d[:, :])
            rs_sb = cpool.tile([CHUNK, nchunk], f32)
            nc.scalar.dma_start(out=rs_sb[:, :], in_=rs_d[:, :])
            io_sb = cpool.tile([CHUNK, 2 * BSLOT], f32)
            nc.scalar.dma_start(out=io_sb[:, :], in_=io_d[:, :])

            fl = None
            for tb in range(nchunk // 4):
                # one DMA loads 4 chunks: DRAM rows (u p) -> SBUF [p, u*W:(u+1)*W]
                xb = xpool.tile([CHUNK, 4 * W], f32r)
                nc.sync.dma_start(
                    out=xb[:, :],
                    in_=xp_d[tb * CHUNK:(tb + 1) * CHUNK, :])
                for u4 in range(4):
                    t = tb * 4 + u4
                    xe = xb[:, u4 * W:(u4 + 1) * W]
                    if t % 2 == 0:
                        bag = bpool.tile([32, 1024], f32)  # [0:346],[512:858]

                    G = gpool.tile([CHUNK, 1024], f32)    # [0:346],[512:856]
                    ohT = oh_sb[:, t * CHUNK:(t + 1) * CHUNK]
                    nc.tensor.matmul(G[:, 0:346], ohT, cw_sb[:, 0:346],
                                     start=True, stop=True)
                    nc.tensor.matmul(G[:, 512:856], ohT, cw_sb[:, 346:DIM],
                                     start=True, stop=True)

                    prod = ppool.tile([CHUNK, DIM], f32)
                    la = spool.tile([CHUNK, 1], f32)
                    lb2 = spool.tile([CHUNK, 1], f32)
                    xv = xe.bitcast(f32)
                    nc.vector.affine_mul_reduce(
                        out=prod[:, 0:346], accum_out=la[:, 0:1],
                        in0=xv[:, 0:346], in1=G[:, 0:346], scale=1.0, bias=0.0)
                    nc.vector.affine_mul_reduce(
                        out=prod[:, 346:DIM], accum_out=lb2[:, 0:1],
                        in0=xv[:, 346:DIM], in1=G[:, 512:856], scale=1.0, bias=0.0)

                    # e = exp(la + lb2); pad rows are all-zero in xe (incl the
                    # ones column) so their e value is irrelevant.
                    e = spool.tile([CHUNK, 1], f32)
                    nc.scalar.activation(e[:, 0:1], la[:, 0:1],
                                         mybir.ActivationFunctionType.Exp,
                                         bias=lb2[:, 0:1], scale=1.0)

                    # two consecutive chunks share one 32-row PSUM block:
                    # even chunk slots 0:16, odd chunk slots 16:32 (host adds
                    # 16 to relseg of odd chunks), accumulated via start/stop.
                    ET = spool.tile([CHUNK, 2 * BSLOT], f32r)
                    nc.vector.tensor_scalar(
                        out=ET[:, :], in0=io_sb[:, :], scalar1=rs_sb[:, t:t + 1],
                        scalar2=e[:, 0:1], op0=mybir.AluOpType.is_equal,
                        op1=mybir.AluOpType.mult)

                    first = (t % 2 == 0)
                    nc.tensor.matmul(bag[0:32, 0:346], ET[:, :], xe[:, 0:346],
                                     start=first, stop=not first)
                    nc.tensor.matmul(bag[0:32, 512:858], ET[:, :],
                                     xe[:, 346:W], start=first, stop=not first)

                    if t % 2 == 1:
                        p = t // 2
                        if p % 4 == 0:
                            fl = fpool.tile([32, 4 * W], f32)
                        # one copy per pair: both PSUM banks via 3D AP
                        nc.scalar.copy(
                            out=fl[:, (p % 4) * W:(p % 4) * W + 692]
                                .rearrange("q (a b) -> q a b", a=2, b=346),
                            in_=bag[0:32, 0:1024]
                                .rearrange("q (a b) -> q a b", a=2, b=512)
                                [:, :, 0:346])
                        if p % 4 == 3:
                            q4 = p // 4
                            dst = tab_d[q4 * 4 * 32:(q4 + 1) * 4 * 32, :]
                            nc.scalar.dma_start(
                                out=dst.rearrange("(u q) d -> q u d", u=4),
                                in_=fl[:, :].rearrange("q (u d) -> q u d", u=4))

    nc.compile()
    return nc


def _prepare(x, rel_weight, att_weight, bias, attention_query, scope):
    x = np.asarray(x, dtype=np.float32)
    rel_weight = np.asarray(rel_weight, dtype=np.float32)
    att_weight = np.asarray(att_weight, dtype=np.float32)
    bias = np.asarray(bias, dtype=np.float32)
    q = np.asarray(attention_query).astype(np.int64)
    scope = np.asarray(scope).astype(np.int64)

    nsent = x.shape[0]
    nbags = len(scope) - 1
    score = nsent // NCORES
    seg = (np.searchsorted(scope, np.arange(nsent), side='right') - 1)
    import ml_dtypes
    cw = (att_weight * rel_weight).astype(ml_dtypes.bfloat16)

    all_chunks = [_pack_core(scope, seg, c * score, (c + 1) * score)
                  for c in range(NCORES)]
    nchunk = max(len(ch) for ch in all_chunks)
    nchunk = (nchunk + 7) // 8 * 8      # device loop needs a multiple of 8
    S = nchunk * CHUNK

    import ml_dtypes
    iota32 = np.ascontiguousarray(
        np.broadcast_to(np.arange(2 * BSLOT, dtype=np.float32), (CHUNK, 2 * BSLOT)))
    in_maps = []
    frag2bag = []
    for c in range(NCORES):
        idx = np.full(S, -1, np.int64)
        relseg = np.zeros(S, np.float32)
        f2b = np.full((nchunk, BSLOT), -1, np.int64)
        for k, ch in enumerate(all_chunks[c]):
            p = k * CHUNK
            for j, (b, s, take) in enumerate(ch):
                idx[p:p + take] = np.arange(s, s + take)
                relseg[p:p + take] = j + BSLOT * (k % 2)
                f2b[k, j] = b
                p += take
        valid = idx >= 0
        xp = np.zeros((S, DIM + 2), np.float32)
        xp[valid, DIM] = 1.0
        xp[valid, :DIM] = x[idx[valid]]
        # pre-block: [nblocks, 4, 128, W] -> [nblocks, 128, 4, W] flat
        xp = np.ascontiguousarray(
            xp.reshape(nchunk // 4, 4, CHUNK, DIM + 2).transpose(0, 2, 1, 3)
        ).reshape((nchunk // 4) * CHUNK, 4 * (DIM + 2))
        qp = np.zeros(S, np.int64)
        qp[valid] = q[idx[valid]]
        oh = (qp[None, :] == np.arange(NCLS)[:, None]).astype(ml_dtypes.bfloat16)
        in_maps.append({
            "xp": xp,
            "oh": np.ascontiguousarray(oh),
            "cw": cw,
            "rs": np.ascontiguousarray(relseg.reshape(nchunk, CHUNK).T),
            "io32": iota32,
        })
        frag2bag.append(f2b)
    return in_maps, frag2bag, nchunk, nbags, rel_weight, bias


def _assemble(tables, frag2bag, nchunk, nbags, rel_weight, bias):
    num = np.zeros((nbags, NCLS))
    den = np.zeros(nbags)
    for c in range(NCORES):
        table = tables[c].reshape(nchunk * BSLOT, DIM + 2)
        U = table[:, :DIM] @ rel_weight.T
        d = table[:, DIM]
        fb = frag2bag[c].ravel()
        v = fb >= 0
        for k in range(NCLS):
            num[:, k] += np.bincount(fb[v], U[v, k], minlength=nbags)
        den += np.bincount(fb[v], d[v], minlength=nbags)
    return (num / den[:, None] + bias[None, :]).astype(np.float32)


def kernel(x, rel_weight, att_weight, bias, attention_query, scope):
    from concourse.bass_utils import run_bass_kernel_spmd

    in_maps, frag2bag, nchunk, nbags, rel, b = _prepare(
        x, rel_weight, att_weight, bias, attention_query, scope)
    if nchunk not in _cache:
        _cache[nchunk] = _build_module(nchunk)
    nc = _cache[nchunk]
    res = run_bass_kernel_spmd(nc, in_maps, list(range(NCORES)))
    tables = [res.results[c]["tab"] for c in range(NCORES)]
    return _assemble(tables, frag2bag, nchunk, nbags, rel, b)
